# revision 1
# baseline (speedup 1.0000x reference)
"""Trainium2 Bass kernel for the 3-layer GAT (nn_GAT_24326694764623).

Strategy (8 NeuronCores, SPMD), v2 (bf16 + transposed aggregation):
  - Nodes sharded 6250/core; edges assigned to the dst's core, grouped by
    dst node-tile (128 nodes), chunked 128 edges/chunk (sorted by src).
  - Per layer: AllGather bf16 node table [h | al_s], per-chunk gather by
    src, dst one-hot masks via tensor_scalar is_equal (pad edges use
    dst=255 -> all-zero mask row), local al_d via transposed-mask matmuls,
    per-tile softmax denominators in PSUM, alpha-weighted transposed
    aggregation (c x (h,d)) straight into PSUM, per-head transform GEMMs,
    global BatchNorm via a tiny AllReduce, relu + residual.
  - All heavy matmuls in bf16 (1 cycle/row vs 4 for fp32).
"""
import os
import sys
import hashlib

for _p in ("/opt/trn_rl_repo", "/root/.axon_site/_ro/trn_rl_repo"):
    if _p not in sys.path:
        sys.path.insert(0, _p)

import numpy as np

N, E = 50000, 400000
HEADS, HID, LAYERS = 4, 128, 3
NCORES = 8
P = 128
NB = N // NCORES            # 6250 real nodes per core
NT = (NB + P - 1) // P      # 49 node tiles per core
NBP = NT * P                # 6272 padded rows per core
NTAB = NCORES * NBP         # 50176 global table rows
TCOL = 256                  # padded table cols [h(128) | al_s(4) | pad] (512B rows)
CCOL = HID + 2 * HEADS      # 136 comb cols [h | al_s | al_d]
HALF = 32768                # int16 gather index limit (table split point)
NEG = 0.2
EPS = 1e-5

_PHASE = int(os.environ.get("GAT_PHASE", "99"))
_COL = int(os.environ.get("GAT_COL", "0"))
_cache = {}


def _wrap16(raw):
    """(...,128) int16 -> (...,128,8) dma_gather wrapped index layout."""
    shp = raw.shape[:-1]
    w = raw.reshape(*shp, 8, 16)
    w = np.swapaxes(w, -1, -2)                    # (...,16,8)
    return np.ascontiguousarray(np.tile(w, (1,) * len(shp) + (8, 1)))


def _build_plan(edge_index):
    src = np.concatenate([edge_index[0].astype(np.int64), np.arange(N, dtype=np.int64)])
    dst = np.concatenate([edge_index[1].astype(np.int64), np.arange(N, dtype=np.int64)])

    # per (core, tile): edge lists sorted by global table row, split at HALF
    per = {}
    nck0 = np.zeros((NCORES, NT), np.int64)
    nck1 = np.zeros((NCORES, NT), np.int64)
    for c in range(NCORES):
        m = (dst >= c * NB) & (dst < (c + 1) * NB)
        s, d = src[m], dst[m] - c * NB
        glob = (s // NB) * NBP + s % NB
        tid = d // P
        for t in range(NT):
            mt = tid == t
            gg, dd = glob[mt], d[mt] - t * P
            o = np.argsort(gg, kind="stable")
            gg, dd = gg[o], dd[o]
            nlo = int(np.searchsorted(gg, HALF))
            per[(c, t)] = (gg, dd, nlo)
            nck0[c, t] = (nlo + P - 1) // P
            nck1[c, t] = (len(gg) - nlo + P - 1) // P
    nk0 = nck0.max(axis=0)
    nk1 = nck1.max(axis=0)
    nks = [(int(a), int(b)) for a, b in zip(nk0, nk1)]   # (lo,hi) chunk counts
    MK = int((nk0 + nk1).max())

    stat_pb = np.full((NCORES, NT, P, MK), 255.0, np.float32)
    idx_raw = np.full((NCORES, NT, MK, P), -1, np.int16)
    for c in range(NCORES):
        for t in range(NT):
            gg, dd, nlo = per[(c, t)]
            k0, _ = nks[t]
            for half, (g_h, d_h, base) in enumerate(
                    [(gg[:nlo], dd[:nlo], 0), (gg[nlo:], dd[nlo:], k0)]):
                off = half * HALF
                nkh = (len(g_h) + P - 1) // P
                for k in range(nkh):
                    sl = slice(k * P, min((k + 1) * P, len(g_h)))
                    n = sl.stop - sl.start
                    stat_pb[c, t, :n, base + k] = d_h[sl].astype(np.float32)
                    idx_raw[c, t, base + k, :n] = (g_h[sl] - off).astype(np.int16)
    # a gather with zero valid indices may never complete -- give pad chunks
    # one harmless row-0 index (its dst stays 255 so it joins nothing)
    empty = (idx_raw == -1).all(axis=-1)
    idx_raw[empty, 0] = 0
    idx_pb = _wrap16(idx_raw)                      # (NCORES, NT, MK, 128, 8)
    idx_pb = np.ascontiguousarray(idx_pb.transpose(0, 1, 3, 2, 4)
                                  ).reshape(NCORES, NT, P, MK * 8)
    # host-precomputed one-hot masks (bf16): m01 (edge x dst), m01T (dst x edge)
    import ml_dtypes
    dl = stat_pb.astype(np.int64)                  # (NC,NT,P,MK) 255 for pad
    m01 = (dl[:, :, :, :, None] == np.arange(P)[None, None, None, None, :])
    m01 = m01.astype(ml_dtypes.bfloat16)           # (NC,NT,P,MK,P)
    m01_pb = np.ascontiguousarray(m01.transpose(0, 1, 2, 3, 4)).reshape(
        NCORES, NT, P, MK * P).view(np.uint16)
    m01T_pb = np.ascontiguousarray(m01.transpose(0, 1, 4, 3, 2)).reshape(
        NCORES, NT, P, MK * P).view(np.uint16)
    return nks, MK, stat_pb, idx_pb, m01_pb, m01T_pb


def _prep_weights(inp):
    import ml_dtypes
    f32, bf16 = np.float32, ml_dtypes.bfloat16
    Wc1 = inp["Wc"][:, :32].astype(f32)
    Wc2 = inp["Wc"][:, 32:].astype(f32)
    W0 = np.concatenate([Wc1, Wc2 @ inp["Wf"].astype(f32)], axis=1)  # (128, 48)
    b0 = inp["bc"].astype(f32) + Wc2 @ inp["bf"].astype(f32)
    vsd = np.zeros((LAYERS, 2 * HEADS, HID), f32)
    Wm = np.zeros((LAYERS, HID, HEADS * HID), f32)  # (c, (h,oc))
    for l in range(LAYERS):
        W = inp["Wl"][l].astype(f32).reshape(HEADS, HID, HID)
        for hh in range(HEADS):
            vsd[l, hh] = W[hh].T @ inp["a_src"][l, hh].astype(f32)
            vsd[l, HEADS + hh] = W[hh].T @ inp["a_dst"][l, hh].astype(f32)
            Wm[l, :, hh * HID:(hh + 1) * HID] = W[hh].T / HEADS  # rhs (c_in x oc)
    gbcol = np.zeros((LAYERS, P, 2), f32)
    gbcol[:, :, 0] = inp["gamma"].astype(f32)
    gbcol[:, :, 1] = inp["beta"].astype(f32)
    return {
        "W0T": np.ascontiguousarray(W0.T).astype(bf16).view(np.uint16),  # (48,128)
        "b0row": b0[None, :].astype(f32),
        "vsdrow": np.ascontiguousarray(vsd.transpose(0, 2, 1)).astype(f32),
        "Wm3": Wm.astype(bf16).view(np.uint16),       # (L, 128, 512)
        "gbcol": gbcol,
        "woutrow": inp["Wout"].astype(f32).copy(),
        "boutsc": np.array([[inp["bout"][0]]], f32),
    }


def _build_nc(nks, MK):
    import concourse.bass as bass
    import concourse.bacc as bacc
    import concourse.mybir as mybir
    import concourse.tile as tile
    import bass_rust as _br
    from contextlib import ExitStack
    from concourse.masks import make_identity

    f32 = mybir.dt.float32
    bf16 = mybir.dt.bfloat16
    i32 = mybir.dt.int32
    Alu = mybir.AluOpType
    Act = mybir.ActivationFunctionType

    i16 = mybir.dt.int16
    nc = bacc.Bacc(None, target_bir_lowering=False, num_swdge_queues=4)
    feat48 = nc.declare_dram_parameter("feat48", [48, NBP], bf16, isOutput=False)
    stat_d = nc.declare_dram_parameter("statpb", [NT, P, MK], f32, isOutput=False)
    idx_d = nc.declare_dram_parameter("idxpb", [NT, P, MK * 8], i16, isOutput=False)
    W0T_d = nc.declare_dram_parameter("W0T", [48, P], bf16, isOutput=False)
    b0_d = nc.declare_dram_parameter("b0row", [1, P], f32, isOutput=False)
    vsd_d = nc.declare_dram_parameter("vsdrow", [LAYERS, HID, 2 * HEADS], f32, isOutput=False)
    m01_d = nc.declare_dram_parameter("m01pb", [NT, P, MK * P], bf16, isOutput=False)
    m01T_d = nc.declare_dram_parameter("m01Tpb", [NT, P, MK * P], bf16, isOutput=False)
    Wm_d = nc.declare_dram_parameter("Wm3", [LAYERS, P, HEADS * HID], bf16, isOutput=False)
    gb_d = nc.declare_dram_parameter("gbcol", [LAYERS, P, 2], f32, isOutput=False)
    wout_d = nc.declare_dram_parameter("woutrow", [1, P], f32, isOutput=False)
    bout_d = nc.declare_dram_parameter("boutsc", [1, 1], f32, isOutput=False)
    out_d = nc.declare_dram_parameter("outp", [NBP, 1], f32, isOutput=True)

    with ExitStack() as ctx:
        tc = ctx.enter_context(tile.TileContext(nc))
        const = ctx.enter_context(tc.tile_pool(name="const", bufs=1))
        big = ctx.enter_context(tc.tile_pool(name="big", bufs=1))
        vtp = ctx.enter_context(tc.tile_pool(name="vtp", bufs=2))
        lw = ctx.enter_context(tc.tile_pool(name="lw", bufs=2))
        stp = ctx.enter_context(tc.tile_pool(name="stp", bufs=6))
        gpool = ctx.enter_context(tc.tile_pool(name="gp", bufs=4))
        mpool = ctx.enter_context(tc.tile_pool(name="mp", bufs=2))
        wpool = ctx.enter_context(tc.tile_pool(name="wp", bufs=3))
        sb = ctx.enter_context(tc.tile_pool(name="sb", bufs=3))
        ps_bt = ctx.enter_context(tc.tile_pool(name="ps_bt", bufs=2, space="PSUM"))
        ps_o = ctx.enter_context(tc.tile_pool(name="ps_o", bufs=2, space="PSUM"))
        ps_sm = ctx.enter_context(tc.tile_pool(name="ps_sm", bufs=2, space="PSUM"))
        ps_c = ctx.enter_context(tc.tile_pool(name="ps_c", bufs=2, space="PSUM"))
        dram = ctx.enter_context(tc.tile_pool(name="dram", bufs=1, space="DRAM"))

        agin = dram.tile([NBP, TCOL], bf16)
        tables = [dram.tile([NTAB, TCOL], bf16, addr_space="Shared", name=f"table{i}")
                  for i in range(LAYERS)]
        bnins = [dram.tile([P, 2], f32, name=f"bnin{i}") for i in range(LAYERS)]
        bnouts = [dram.tile([P, 2], f32, addr_space="Shared", name=f"bnout{i}")
                  for i in range(LAYERS)]
        rowbuf = dram.tile([P, 2], f32)

        # ---------------- constants ----------------
        iota_bf = const.tile([P, P], bf16)
        nc.gpsimd.iota(iota_bf[:], pattern=[[1, P]], base=0, channel_multiplier=0,
                       allow_small_or_imprecise_dtypes=True)
        ident_f = const.tile([P, P], f32)
        make_identity(nc, ident_f[:])
        ones_col = const.tile([P, 1], f32)
        nc.vector.memset(ones_col[:], 1.0)
        ones1_bf = const.tile([1, P], bf16)
        nc.vector.memset(ones1_bf[:], 1.0)
        rows3 = const.tile([1, 3 * P], f32)
        nc.sync.dma_start(out=rows3[:, 0:P], in_=b0_d[:1, :])
        nc.sync.dma_start(out=rows3[:, P:2 * P], in_=wout_d[:1, :])
        nc.sync.dma_start(out=rows3[:, 2 * P:2 * P + 1], in_=bout_d[:1, :1])
        b0_bc = const.tile([P, P], f32)
        nc.gpsimd.partition_broadcast(b0_bc[:], rows3[:1, 0:P])
        wout_bc = const.tile([P, P], f32)
        nc.gpsimd.partition_broadcast(wout_bc[:], rows3[:1, P:2 * P])
        bout_col = const.tile([P, 1], f32)
        nc.gpsimd.partition_broadcast(bout_col[:], rows3[:1, 2 * P:2 * P + 1])
        W0T_t = const.tile([48, P], bf16)
        nc.sync.dma_start(out=W0T_t[:], in_=W0T_d[:])

        hbufA = big.tile([P, NT * P], f32)
        hbufB = big.tile([P, NT * P], f32)
        ald_bufs = [big.tile([P, NT * 2 * HEADS], bf16, name=f"ald{i}") for i in range(2)]

        # zero all rotations of the gather pool: rows skipped by the gather
        # (idx=-1 pad slots) must read as finite values, never boot garbage
        for _ in range(4):
            gz = gpool.tile([P, MK * TCOL], bf16, tag="G")
            nc.vector.memset(gz[:], 0.0)

        def build_vt8(l):
            """vsd[l] as (c x 8) f32 rhs for the per-tile al matmul."""
            vsd8 = vtp.tile([P, 2 * HEADS], f32, tag="vsd8")
            nc.sync.dma_start(out=vsd8[:], in_=vsd_d[l, :, :])
            return vsd8

        agin_writes = []

        def comb_and_table(hsl, t, vsd8, ald_next):
            """From post-activation hsl (f32, node x 128): build comb block
            [h bf16 x128 | al_s f32x4 (bitcast)], stash al_d hi/lo bf16.
            al computed on tensor: hT = hsl^T, al8 = hT.T @ vsd8 (fp32)."""
            comb = sb.tile([P, CCOL], bf16, tag="comb")
            nc.scalar.copy(out=comb[:, 0:HID], in_=hsl)
            psC = ps_c.tile([P, HID + 2 * HEADS], f32, space="PSUM", tag="psC")
            nc.tensor.transpose(out=psC[:, 0:HID], in_=hsl, identity=ident_f[:])
            hT = sb.tile([P, HID], f32, tag="hT")
            nc.scalar.copy(out=hT[:], in_=psC[:, 0:HID])
            nc.tensor.matmul(out=psC[:, HID:HID + 2 * HEADS], lhsT=hT[:],
                             rhs=vsd8[:], start=True, stop=True)
            nc.scalar.copy(out=comb[:, HID:HID + 2 * HEADS].bitcast(f32),
                           in_=psC[:, HID:HID + HEADS])
            hi = ald_next[:, t * 2 * HEADS:t * 2 * HEADS + HEADS]
            nc.scalar.copy(out=hi, in_=psC[:, HID + HEADS:HID + 2 * HEADS])
            nc.vector.tensor_tensor(
                out=ald_next[:, t * 2 * HEADS + HEADS:(t + 1) * 2 * HEADS],
                in0=psC[:, HID + HEADS:HID + 2 * HEADS], in1=hi, op=Alu.subtract)
            d = nc.sync.dma_start(out=agin[t * P:(t + 1) * P, 0:HID + 2 * HEADS],
                                  in_=comb[:, 0:HID + 2 * HEADS])
            agin_writes.append(d)

        # ---------------- encoder ----------------
        vt8_cur = build_vt8(0)
        for t in range(NT):
            lhs48 = sb.tile([48, P], bf16, tag="lhs48")
            nc.sync.dma_start(out=lhs48[:], in_=feat48[:, t * P:(t + 1) * P])
            pseq = ps_o.tile([P, HEADS * P], f32, space="PSUM", tag="psoq")
            pse = pseq[:, 0:P]
            nc.tensor.matmul(out=pse[:], lhsT=lhs48[:], rhs=W0T_t[:], start=True, stop=True)
            hsl = hbufA[:, t * P:(t + 1) * P]
            nc.vector.tensor_tensor(out=hsl, in0=pse[:], in1=b0_bc[:], op=Alu.add)
            nc.vector.tensor_scalar_max(out=hsl, in0=hsl, scalar1=0.0)
            comb_and_table(hsl, t, vt8_cur, ald_bufs[0])
        cc = nc.gpsimd.collective_compute(
            "AllGather", Alu.bypass, replica_groups=[list(range(NCORES))],
            ins=[agin.opt()], outs=[tables[0].opt()])
        for d in agin_writes:
            _br.add_dep_helper(cc.ins, d.ins, sync=True, reason="AG after agin writes")
        agin_writes.clear()

        if _PHASE == 0:
            for t in range(NT):
                nc.sync.dma_start(out=out_d[t * P:(t + 1) * P, :],
                                  in_=hbufA[:, t * P + _COL:t * P + _COL + 1])
        # ---------------- layers ----------------
        for l in range(LAYERS if _PHASE > 0 else 0):
            hprev = hbufA if l % 2 == 0 else hbufB
            hpre = hbufB if l % 2 == 0 else hbufA
            ald_cur = ald_bufs[l % 2]
            ald_next = ald_bufs[(l + 1) % 2]
            wm_t = lw.tile([P, HEADS * HID], bf16, tag="wm")
            nc.sync.dma_start(out=wm_t[:], in_=Wm_d[l, :, :])
            gb_t = lw.tile([P, 2], f32, tag="gb")
            nc.sync.dma_start(out=gb_t[:], in_=gb_d[l, :, :])

            stats_sb = sb.tile([P, 2], f32, tag="statsb")
            last_gather = None
            tab_lo = tables[l][0:HALF, :]
            tab_hi = tables[l][HALF:NTAB, :]

            for t in range(NT):
                nk0, nk1 = nks[t]
                nk = nk0 + nk1
                stat = stp.tile([P, MK], f32, tag="stat")
                nc.sync.dma_start(out=stat[:], in_=stat_d[t, :, :])
                sidxt = stp.tile([P, MK * 8], i16, tag="sidx")
                nc.sync.dma_start(out=sidxt[:], in_=idx_d[t, :, :])
                m01t = mpool.tile([P, MK * P], bf16, tag="m01")
                nc.sync.dma_start(out=m01t[:, 0:nk * P], in_=m01_d[t, :, 0:nk * P])
                m01Tt = mpool.tile([P, MK * P], bf16, tag="m01T")
                nc.sync.dma_start(out=m01Tt[:, 0:nk * P], in_=m01T_d[t, :, 0:nk * P])
                Gt = gpool.tile([P, MK * TCOL], bf16, tag="G")
                psS = ps_sm.tile([P, 128], f32, space="PSUM", tag="psS")
                psAL = psS[:, 8:8 + MK * 2 * HEADS]
                psD = psS[:, 0:HEADS]
                stps = psS[:, 4:6]

                for j in range(nk):
                    gi = nc.gpsimd.dma_gather(
                        Gt[:, None, j * TCOL:(j + 1) * TCOL],
                        tab_lo if j < nk0 else tab_hi,
                        sidxt[:, j * 8:(j + 1) * 8], P, P, TCOL,
                        queue_num=(t * MK + j) % 4)
                    _br.add_dep_helper(gi.ins, cc.ins, sync=True, reason="gather after AG")
                    last_gather = gi
                    nc.tensor.matmul(
                        out=psAL[:, j * 2 * HEADS:(j + 1) * 2 * HEADS],
                        lhsT=m01Tt[:, j * P:(j + 1) * P],
                        rhs=ald_cur[:, t * 2 * HEADS:(t + 1) * 2 * HEADS],
                        start=True, stop=True)

                eat = sb.tile([P, MK * HEADS], f32, tag="eat")
                nc.vector.tensor_tensor(
                    out=eat[:, 0:nk * HEADS].rearrange("p (k h) -> p k h", k=nk),
                    in0=Gt[:, 0:nk * TCOL].rearrange(
                        "p (k c) -> p k c", k=nk)[:, :, HID:HID + 2 * HEADS].bitcast(f32),
                    in1=psAL[:, 0:nk * 2 * HEADS].rearrange(
                        "p (k h) -> p k h", k=nk)[:, :, 0:HEADS],
                    op=Alu.add)
                nc.vector.tensor_tensor(
                    out=eat[:, 0:nk * HEADS].rearrange("p (k h) -> p k h", k=nk),
                    in0=eat[:, 0:nk * HEADS].rearrange("p (k h) -> p k h", k=nk),
                    in1=psAL[:, 0:nk * 2 * HEADS].rearrange(
                        "p (k h) -> p k h", k=nk)[:, :, HEADS:2 * HEADS],
                    op=Alu.add)
                nc.vector.scalar_tensor_tensor(
                    out=eat[:, 0:nk * HEADS], in0=eat[:, 0:nk * HEADS], scalar=NEG,
                    in1=eat[:, 0:nk * HEADS], op0=Alu.mult, op1=Alu.max)
                exf = sb.tile([P, MK * HEADS], f32, tag="exf")
                nc.scalar.activation(out=exf[:, 0:nk * HEADS], in_=eat[:, 0:nk * HEADS],
                                     func=Act.Exp)
                exb = sb.tile([P, MK * HEADS], bf16, tag="exb")
                nc.scalar.copy(out=exb[:, 0:nk * HEADS], in_=exf[:, 0:nk * HEADS])

                psBT = ps_bt.tile([P, HEADS * P], f32, space="PSUM", tag="psBT")
                for j in range(nk):
                    nc.tensor.matmul(out=psD[:], lhsT=m01t[:, j * P:(j + 1) * P],
                                     rhs=exb[:, j * HEADS:(j + 1) * HEADS],
                                     start=(j == 0), stop=(j == nk - 1))
                    Wt = wpool.tile([P, HEADS * P], bf16, tag="W")
                    if (t * MK + j) % 5 < 3:
                        nc.vector.scalar_tensor_tensor(
                            out=Wt[:].rearrange("p (h d) -> p h d", h=HEADS),
                            in0=iota_bf[:, None, :].to_broadcast([P, HEADS, P]),
                            scalar=stat[:, j:j + 1],
                            in1=exb[:, j * HEADS:(j + 1) * HEADS][:, :, None].to_broadcast(
                                [P, HEADS, P]),
                            op0=Alu.is_equal, op1=Alu.mult)
                    else:
                        for hh in range(HEADS):
                            nc.scalar.activation(
                                out=Wt[:, hh * P:(hh + 1) * P],
                                in_=m01t[:, j * P:(j + 1) * P], func=Act.Copy,
                                scale=exf[:, j * HEADS + hh:j * HEADS + hh + 1])
                    nc.tensor.matmul(out=psBT[:], lhsT=Gt[:, j * TCOL:j * TCOL + HID],
                                     rhs=Wt[:], start=(j == 0), stop=(j == nk - 1))

                den_sb = sb.tile([P, HEADS], f32, tag="densb")
                nc.vector.tensor_scalar_add(out=den_sb[:], in0=psD[:], scalar1=1e-16)
                lnd = sb.tile([P, HEADS], f32, tag="lnd")
                nc.scalar.activation(out=lnd[:], in_=den_sb[:], func=Act.Ln)
                rden = sb.tile([P, HEADS], f32, tag="rden")
                nc.scalar.activation(out=rden[:], in_=lnd[:], func=Act.Exp, scale=-1.0)
                # one Newton step: r <- r * (2 - den * r)
                nrt = sb.tile([P, HEADS], f32, tag="nrt")
                nc.vector.tensor_tensor(out=nrt[:], in0=den_sb[:], in1=rden[:], op=Alu.mult)
                nc.vector.tensor_scalar(out=nrt[:], in0=nrt[:], scalar1=-1.0,
                                        scalar2=2.0, op0=Alu.mult, op1=Alu.add)
                nc.vector.tensor_tensor(out=rden[:], in0=rden[:], in1=nrt[:], op=Alu.mult)
                normBT = wpool.tile([P, HEADS * P], bf16, tag="normBT")
                nc.scalar.copy(out=normBT[:], in_=psBT[:])
                psoq = ps_o.tile([P, HEADS * P], f32, space="PSUM", tag="psoq")
                for hh in range(HEADS):
                    nc.tensor.matmul(out=psoq[:, hh * P:(hh + 1) * P],
                                     lhsT=normBT[:, hh * P:(hh + 1) * P],
                                     rhs=wm_t[:, hh * P:(hh + 1) * P],
                                     start=True, stop=True)
                hsl = hpre[:, t * P:(t + 1) * P]
                nc.vector.tensor_scalar(
                    out=hsl, in0=psoq[:, 0:P], scalar1=rden[:, 0:1],
                    scalar2=None, op0=Alu.mult)
                for hh in range(1, HEADS):
                    nc.vector.scalar_tensor_tensor(
                        out=hsl, in0=psoq[:, hh * P:(hh + 1) * P],
                        scalar=rden[:, hh:hh + 1], in1=hsl,
                        op0=Alu.mult, op1=Alu.add)
                nrow = min(P, NB - t * P)
                sq = sb.tile([P, P], f32, tag="sq")
                nc.vector.tensor_tensor(out=sq[:nrow, :], in0=hsl[:nrow, :],
                                        in1=hsl[:nrow, :], op=Alu.mult)
                nc.tensor.matmul(out=stps[:, 0:1], lhsT=hsl[:nrow, :],
                                 rhs=ones_col[:nrow, :], start=True, stop=True)
                nc.tensor.matmul(out=stps[:, 1:2], lhsT=sq[:nrow, :],
                                 rhs=ones_col[:nrow, :], start=True, stop=True)
                if t == 0:
                    nc.vector.tensor_copy(out=stats_sb[:], in_=stps[:])
                else:
                    nc.vector.tensor_tensor(out=stats_sb[:], in0=stats_sb[:],
                                            in1=stps[:], op=Alu.add)

            if _PHASE == 2 * l + 1:
                for t in range(NT):
                    nc.sync.dma_start(out=out_d[t * P:(t + 1) * P, :],
                                      in_=hpre[:, t * P + _COL:t * P + _COL + 1])
                break
            # ---- BN stats -> AllReduce -> scale/shift ----
            d_bn = nc.sync.dma_start(out=bnins[l][:, :], in_=stats_sb[:])
            ar = nc.gpsimd.collective_compute(
                "AllReduce", Alu.add, replica_groups=[list(range(NCORES))],
                ins=[bnins[l].opt()], outs=[bnouts[l].opt()])
            _br.add_dep_helper(ar.ins, d_bn.ins, sync=True, reason="AR after stats write")
            st2 = sb.tile([P, 2], f32, tag="st2")
            d_ar = nc.sync.dma_start(out=st2[:], in_=bnouts[l][:, :])
            _br.add_dep_helper(d_ar.ins, ar.ins, sync=True, reason="read after AR")
            mu = sb.tile([P, 1], f32, tag="mu")
            nc.vector.tensor_scalar_mul(out=mu[:], in0=st2[:, 0:1], scalar1=1.0 / N)
            var = sb.tile([P, 1], f32, tag="var")
            nc.vector.tensor_scalar_mul(out=var[:], in0=st2[:, 1:2], scalar1=1.0 / N)
            musq = sb.tile([P, 1], f32, tag="musq")
            nc.vector.tensor_tensor(out=musq[:], in0=mu[:], in1=mu[:], op=Alu.mult)
            nc.vector.tensor_tensor(out=var[:], in0=var[:], in1=musq[:], op=Alu.subtract)
            nc.vector.tensor_scalar_add(out=var[:], in0=var[:], scalar1=EPS)
            rstd = sb.tile([P, 1], f32, tag="rstd")
            nc.scalar.sqrt(out=rstd[:], in_=var[:])
            nc.vector.reciprocal(out=rstd[:], in_=rstd[:])
            ssc = sb.tile([P, 2], f32, tag="ssc")
            nc.vector.tensor_tensor(out=ssc[:, 0:1], in0=gb_t[:, 0:1], in1=rstd[:], op=Alu.mult)
            nc.vector.tensor_tensor(out=musq[:], in0=mu[:], in1=ssc[:, 0:1], op=Alu.mult)
            nc.vector.tensor_tensor(out=ssc[:, 1:2], in0=gb_t[:, 1:2], in1=musq[:], op=Alu.subtract)
            nc.sync.dma_start(out=rowbuf[:, :], in_=ssc[:])
            srow = sb.tile([1, P], f32, tag="srow")
            nc.sync.dma_start(out=srow[:], in_=rowbuf[:, 0:1].rearrange("p c -> c p"))
            hrow = sb.tile([1, P], f32, tag="hrow")
            nc.sync.dma_start(out=hrow[:], in_=rowbuf[:, 1:2].rearrange("p c -> c p"))
            scale_bc = sb.tile([P, P], f32, tag="scalebc")
            nc.gpsimd.partition_broadcast(scale_bc[:], srow[:1, :])
            shift_bc = sb.tile([P, P], f32, tag="shiftbc")
            nc.gpsimd.partition_broadcast(shift_bc[:], hrow[:1, :])

            # ---- apply BN + relu + residual (+ next table / final head) ----
            vt8_next = build_vt8(l + 1) if l + 1 < LAYERS else None
            for t in range(NT):
                hsl = hpre[:, t * P:(t + 1) * P]
                hpv = hprev[:, t * P:(t + 1) * P]
                tmp = sb.tile([P, P], f32, tag="applytmp")
                nc.vector.tensor_tensor(out=tmp[:], in0=hsl, in1=scale_bc[:], op=Alu.mult)
                nc.vector.tensor_tensor(out=tmp[:], in0=tmp[:], in1=shift_bc[:], op=Alu.add)
                nc.vector.scalar_tensor_tensor(out=hsl, in0=tmp[:], scalar=0.0,
                                               in1=hpv, op0=Alu.max, op1=Alu.add)
                if l + 1 < LAYERS:
                    comb_and_table(hsl, t, vt8_next, ald_next)
                else:
                    scr2 = sb.tile([P, P], f32, tag="scr_f")
                    ocol = sb.tile([P, 1], f32, tag="ocol")
                    nc.vector.tensor_tensor(out=scr2[:], in0=hsl, in1=wout_bc[:], op=Alu.mult)
                    nc.vector.tensor_reduce(out=ocol[:], in_=scr2[:],
                                            op=Alu.add, axis=mybir.AxisListType.X)
                    nc.vector.tensor_tensor(out=ocol[:], in0=ocol[:], in1=bout_col[:], op=Alu.add)
                    nc.vector.tensor_scalar_min(out=ocol[:], in0=ocol[:], scalar1=10.0)
                    nc.vector.tensor_scalar_max(out=ocol[:], in0=ocol[:], scalar1=-10.0)
                    nc.sync.dma_start(out=out_d[t * P:(t + 1) * P, :], in_=ocol[:])
            if _PHASE == 2 * l + 2:
                for t in range(NT):
                    nc.sync.dma_start(out=out_d[t * P:(t + 1) * P, :],
                                      in_=hpre[:, t * P + _COL:t * P + _COL + 1])
                break
            if l + 1 < LAYERS:
                cc = nc.gpsimd.collective_compute(
                    "AllGather", Alu.bypass, replica_groups=[list(range(NCORES))],
                    ins=[agin.opt()], outs=[tables[l + 1].opt()])
                _br.add_dep_helper(cc.ins, last_gather.ins, sync=True, reason="AG after gathers")
                for d in agin_writes:
                    _br.add_dep_helper(cc.ins, d.ins, sync=True, reason="AG after agin writes")
                agin_writes.clear()

    nc.compile()
    return nc


def _get_compiled(edge_index):
    key = (hashlib.md5(np.ascontiguousarray(edge_index).tobytes()).hexdigest(),
           _PHASE, _COL)
    if key not in _cache:
        nks, MK, stat_pb, idx_pb, m01_pb, m01T_pb = _build_plan(edge_index)
        nc = _build_nc(nks, MK)
        _cache[key] = (nc, stat_pb, idx_pb, m01_pb, m01T_pb)
    return _cache[key]


def _make_in_maps(inputs, stat_pb, idx_pb, m01_pb, m01T_pb):
    import ml_dtypes
    w = _prep_weights(inputs)
    x = inputs["x"].astype(np.float32)
    emb = inputs["emb"].astype(np.float32)
    in_maps = []
    for c in range(NCORES):
        f48 = np.zeros((48, NBP), np.float32)
        blk = slice(c * NB, (c + 1) * NB)
        f48[:32, :NB] = emb[blk].T
        f48[32:, :NB] = x[blk].T
        m = {"feat48": f48.astype(ml_dtypes.bfloat16).view(np.uint16),
             "statpb": stat_pb[c], "idxpb": idx_pb[c],
             "m01pb": m01_pb[c], "m01Tpb": m01T_pb[c]}
        m.update(w)
        in_maps.append(m)
    return in_maps


def kernel(**inputs):
    from concourse.bass_utils import run_bass_kernel_spmd
    nc, stat_pb, idx_pb, m01_pb, m01T_pb = _get_compiled(np.asarray(inputs["edge_index"]))
    in_maps = _make_in_maps(inputs, stat_pb, idx_pb, m01_pb, m01T_pb)
    res = run_bass_kernel_spmd(nc, in_maps, list(range(NCORES)))
    out = np.concatenate([res.results[c]["outp"][:NB] for c in range(NCORES)], axis=0)
    return out.astype(np.float32)

